# revision 14
# baseline (speedup 1.0000x reference)
"""Cross-attention kernel for Trainium2, 8 NeuronCores.

Problem (hardcoded): x[4,2048,1024], cond[4,1024,768], dim=1024, cond_dim=768,
H=16 heads, hd=64.  out = proj(softmax(q k^T / sqrt(hd)) v) + proj_b.

Sharding: Megatron-style hybrid — batch (4) x head-half (2) = 8 shards.
Core c handles batch b=c//2 and heads [8*(c%2), 8*(c%2)+8).  Each core
computes its 8 heads' attention and a partial projection output [2048,1024];
the host sums the two partials per batch and adds the biases folded out of
the device program (proj_b and the v-bias term kv_b_v @ proj_w).

Device math per core (all matmuls in float32r, 1 PE cycle/row):
  qT  = (x W_q + q_b)^T          [512, 2048]   (pairs of heads = 128-row chunks)
  kT  = (cond W_k + k_b)^T       [512, 1024]
  v   =  cond W_v                [1024, 512]   (v-bias folded to host)
  per head pair p, query span s (512 tokens), cond chunk mc (128 tokens):
    scores^T = row-packed pair of K=64 matmuls -> psum [128 m, 2 heads, 512 nq]
    exp      = one ACT op: exp(0.125 * scores) -> f32r
    AV+sums  = K=128 matmul with lhsT = [v_h | ones] (even) / [ones | v_h] (odd)
               accumulated over mc -> [64 av rows, 64 sum rows]
    out_h    = av * reciprocal(sums)
  partial = outT^T @ W_p_slice   [2048, 1024]  (accumulate 4 pairs in psum)
"""

import sys

if '/opt/trn_rl_repo' not in sys.path:
    sys.path.insert(0, '/opt/trn_rl_repo')

import numpy as np

B, N, C = 4, 2048, 1024
CONDN, CONDC = 1024, 768
H, HD = 16, 64
N_CORES = 8
SCALE = HD ** -0.5

NPAIR = 4            # head pairs per core (8 heads)
NSPAN = 4            # query spans of 512
SPAN = 512
NMC = 8              # cond chunks of 128
KC_Q = 8             # contraction chunks for q proj (1024/128)
KC_KV = 6            # contraction chunks for kv proj (768/128)

_COMPILED = None
LAST_RESULTS = None


def _build():
    import concourse.bacc as bacc
    import concourse.mybir as mybir
    from concourse import tile

    F32R = mybir.dt.float32r
    F32 = mybir.dt.float32
    MULT = mybir.AluOpType.mult
    EXP = mybir.ActivationFunctionType.Exp

    nc = bacc.Bacc("TRN2", target_bir_lowering=False, num_devices=N_CORES)

    xT_d = nc.dram_tensor("xT", [C, N], F32R, kind="ExternalInput")
    condT_d = nc.dram_tensor("condT", [CONDC, CONDN], F32R, kind="ExternalInput")
    qw_d = nc.dram_tensor("qw", [C, 512], F32R, kind="ExternalInput")
    kwk_d = nc.dram_tensor("kwk", [CONDC, 512], F32R, kind="ExternalInput")
    kwv_d = nc.dram_tensor("kwv", [CONDC, 512], F32R, kind="ExternalInput")
    pw_d = nc.dram_tensor("pw", [512, C], F32R, kind="ExternalInput")
    qb_d = nc.dram_tensor("qb", [512], F32, kind="ExternalInput")
    kb_d = nc.dram_tensor("kb", [512], F32, kind="ExternalInput")
    vones_d = nc.dram_tensor("vones_init", [128, NMC, 8, 128], F32R, kind="ExternalInput")
    out_d = nc.dram_tensor("out", [N, C], F32, kind="ExternalOutput")

    with tile.TileContext(nc) as tc:
        with (
            tc.tile_pool(name="const", bufs=1) as const,
            tc.tile_pool(name="xt", bufs=16) as xt_pool,
            tc.tile_pool(name="ct", bufs=6) as ct_pool,
            tc.tile_pool(name="qt", bufs=2) as qt_pool,
            tc.tile_pool(name="ex", bufs=3) as ex_pool,
            tc.tile_pool(name="ot", bufs=2) as ot_pool,
            tc.tile_pool(name="rc", bufs=2) as rc_pool,
            tc.tile_pool(name="ob", bufs=3) as ob_pool,
            tc.tile_pool(name="psA", bufs=2, space="PSUM") as psA,
            tc.tile_pool(name="psB", bufs=3, space="PSUM") as psB,
            tc.tile_pool(name="psC", bufs=1, space="PSUM") as psC,
        ):
            # ---- persistent weights / constants -------------------------------
            # DMA emission order = consumption order, so nothing early waits
            # on a large transfer it does not need: kv weights first (KV phase
            # runs first), vones chunks (needed mid-KV-phase), q weights
            # (Q-proj after KV), proj weights last (first use is in span 1).
            kwk_sb = const.tile([128, KC_KV, 512], F32R)
            nc.sync.dma_start(kwk_sb[:], kwk_d.ap().rearrange("(kc p) m -> p kc m", p=128))
            kb_sb = const.tile([128, NPAIR], F32)
            nc.sync.dma_start(kb_sb[:], kb_d.ap().rearrange("(pp p) -> p pp", p=128))
            kwv_sb = const.tile([128, KC_KV, 512], F32R)
            nc.sync.dma_start(kwv_sb[:], kwv_d.ap().rearrange("(kc p) m -> p kc m", p=128))
            vones = const.tile([128, NMC, 8, 128], F32R)
            for mc in range(NMC):
                nc.sync.dma_start(vones[:, mc], vones_d.ap()[:, mc])
            qw_sb = const.tile([128, KC_Q, 512], F32R)
            qb_sb = const.tile([128, NPAIR], F32)
            pw_sb = const.tile([128, NPAIR, C], F32R)
            kT_sb = const.tile([128, NPAIR, NMC, 128], F32R)

            # ---- KV phase: kT = (cond Wk + kb)^T, v = cond Wv ----------------
            for ms in range(2):  # cond-token spans of 512
                cts = []
                for kc in range(KC_KV):
                    ct = ct_pool.tile([128, 512], F32R, name="ct")
                    nc.sync.dma_start(
                        ct[:],
                        condT_d.ap()[kc * 128:(kc + 1) * 128, ms * 512:(ms + 1) * 512],
                    )
                    cts.append(ct)
                for p in range(NPAIR):
                    ps = psC.tile([128, 512], F32, tag="C", name="kps")
                    for kc in range(KC_KV):
                        nc.tensor.matmul(
                            ps[:], kwk_sb[:, kc, p * 128:(p + 1) * 128], cts[kc][:],
                            start=(kc == 0), stop=(kc == KC_KV - 1),
                        )
                    nc.vector.tensor_scalar_add(
                        kT_sb[:, p, ms * 4:(ms + 1) * 4, :], ps[:], kb_sb[:, p:p + 1],
                    )
                for mj in range(4):  # token chunks of 128 within span
                    mc = ms * 4 + mj
                    ps = psC.tile([128, 512], F32, tag="C", name="vps")
                    for kc in range(KC_KV):
                        nc.tensor.matmul(
                            ps[:], cts[kc][:, mj * 128:(mj + 1) * 128], kwv_sb[:, kc, :],
                            start=(kc == 0), stop=(kc == KC_KV - 1),
                        )
                    # scatter v into [v|1]/[1|v] interleave: even heads cols 0:64,
                    # odd heads cols 64:128
                    ps_v = ps.rearrange("q (h d) -> q h d", d=64)
                    nc.vector.tensor_copy(vones[:, mc, 0::2, 0:64], ps_v[:, 0::2, :])
                    nc.vector.tensor_copy(vones[:, mc, 1::2, 64:128], ps_v[:, 1::2, :])

            # ---- main loop over query spans ----------------------------------
            # PE stream is kept dense three ways: AV matmuls trail their QK by
            # one cond-chunk (so the exp they wait on has a chunk of slack),
            # Q-proj groups for span s+1 and output-proj groups for span s-1
            # are spread into span s's attention loop as filler work.
            def dma_xts(s):
                xts = []
                for kc in range(KC_Q):
                    xt = xt_pool.tile([128, 512], F32R, name="xt")
                    nc.sync.dma_start(
                        xt[:],
                        xT_d.ap()[kc * 128:(kc + 1) * 128, s * SPAN:(s + 1) * SPAN],
                    )
                    xts.append(xt)
                return xts

            def qproj_pair(qt, xts, p):
                """Returns a list of single-matmul thunks (filler granularity)."""
                cell = {}

                def mm(kc):
                    if kc == 0:
                        cell["ps"] = psC.tile([128, 512], F32, tag="C", name="qps")
                    nc.tensor.matmul(
                        cell["ps"][:], qw_sb[:, kc, p * 128:(p + 1) * 128], xts[kc][:],
                        start=(kc == 0), stop=(kc == KC_Q - 1),
                    )
                    if kc == KC_Q - 1:
                        nc.vector.tensor_scalar_add(
                            qt[:, p, :], cell["ps"][:], qb_sb[:, p:p + 1],
                        )

                return [lambda kc=kc: mm(kc) for kc in range(KC_Q)]

            def proj_group(s, ot, t, o):
                cell = {}

                def mm(p):
                    if p == 0:
                        cell["pp"] = psC.tile([128, 512], F32, tag="C", name="pp")
                    nc.tensor.matmul(
                        cell["pp"][:], ot[:, p, t * 128:(t + 1) * 128],
                        pw_sb[:, p, o * 512:(o + 1) * 512],
                        start=(p == 0), stop=(p == NPAIR - 1),
                    )
                    if p == NPAIR - 1:
                        ob = ob_pool.tile([128, 512], F32, name="ob")
                        nc.vector.tensor_copy(ob[:], cell["pp"][:])
                        nc.sync.dma_start(
                            out_d.ap()[s * SPAN + t * 128:s * SPAN + (t + 1) * 128,
                                       o * 512:(o + 1) * 512],
                            ob[:],
                        )

                return [lambda p=p: mm(p) for p in range(NPAIR)]

            nc.sync.dma_start(qw_sb[:], qw_d.ap().rearrange("(kc p) m -> p kc m", p=128))
            nc.sync.dma_start(qb_sb[:], qb_d.ap().rearrange("(pp p) -> p pp", p=128))
            xts = dma_xts(0)
            qt = qt_pool.tile([128, NPAIR, SPAN], F32R, name="qt")
            for p in range(NPAIR):
                for th in qproj_pair(qt, xts, p):
                    th()
            nc.sync.dma_start(pw_sb[:], pw_d.ap().rearrange("(pp p) o -> p pp o", p=128))

            prev_ot = None
            for s in range(NSPAN):
                # filler matmuls to interleave into this span's attention loop
                # (one thunk per matmul): Q-proj for span s+1 and output-proj
                # for span s-1.
                filler = []
                if s + 1 < NSPAN:
                    next_xts = dma_xts(s + 1)
                    next_qt = qt_pool.tile([128, NPAIR, SPAN], F32R, name="qt")
                    for p in range(NPAIR):
                        filler.extend(qproj_pair(next_qt, next_xts, p))
                else:
                    next_xts = next_qt = None
                if prev_ot is not None:
                    for t in range(4):
                        for o in range(2):
                            filler.extend(proj_group(s - 1, prev_ot, t, o))
                filler.reverse()  # pop() from the front
                per_step = max(1, -(-len(filler) // 32))  # ceil over 32 mc-steps

                ot = ot_pool.tile([128, NPAIR, SPAN], F32R, name="ot")
                for p in range(NPAIR):
                    av = [psB.tile([128, 512], F32, tag="av", name=f"av{h}") for h in range(2)]
                    pend = []  # (mc, exp tile) entries awaiting their AV matmuls
                    for mc in range(NMC):
                        qk = psA.tile([128, 1024], F32, tag="A", name="qk")
                        qk2 = qk.rearrange("q (h n) -> q h n", h=2)
                        nc.tensor.matmul(
                            qk2[:, 0], kT_sb[0:64, p, mc, :], qt[0:64, p, :],
                            start=True, stop=True,
                        )
                        nc.tensor.matmul(
                            qk2[:, 1], kT_sb[64:128, p, mc, :], qt[64:128, p, :],
                            start=True, stop=True,
                        )
                        ex = ex_pool.tile([128, 2, 512], F32R, name="ex")
                        nc.scalar.activation(ex[:], qk2[:], EXP, scale=SCALE)
                        for _ in range(per_step):
                            if filler:
                                filler.pop()()
                        pend.append((mc, ex))
                        if len(pend) > 2:  # AV trails its QK by two cond-chunks
                            pmc, pex = pend.pop(0)
                            for h in range(2):
                                nc.tensor.matmul(
                                    av[h][:], vones[:, pmc, 2 * p + h, :], pex[:, h, :],
                                    start=(pmc == 0), stop=False,
                                )
                    for pmc, pex in pend:
                        for h in range(2):
                            nc.tensor.matmul(
                                av[h][:], vones[:, pmc, 2 * p + h, :], pex[:, h, :],
                                start=(pmc == 0), stop=(pmc == NMC - 1),
                            )
                    # normalize: even head -> av rows 0:64, sums 64:128;
                    #            odd head  -> sums 0:64, av 64:128
                    sums = rc_pool.tile([128, 512], F32, name="sums")
                    nc.vector.tensor_copy(sums[0:64, :], av[0][64:128, :])
                    nc.vector.tensor_copy(sums[64:128, :], av[1][0:64, :])
                    rcp = rc_pool.tile([128, 512], F32, name="rcp")
                    # approx_fast needs SBUF input (fp32 bit-trick seed), ~51 ULP
                    nc.vector.reciprocal_approx_fast(rcp[:], sums[:])
                    nc.vector.tensor_tensor(
                        ot[0:64, p, :], av[0][0:64, :], rcp[0:64, :], op=MULT,
                    )
                    nc.vector.tensor_tensor(
                        ot[64:128, p, :], av[1][64:128, :], rcp[64:128, :], op=MULT,
                    )
                while filler:
                    filler.pop()()
                prev_ot = ot
                qt = next_qt

            # trailing projection for the last span
            for t in range(4):
                for o in range(2):
                    for th in proj_group(NSPAN - 1, prev_ot, t, o):
                        th()

    nc.compile()
    return nc


def _get_compiled():
    global _COMPILED
    if _COMPILED is None:
        _COMPILED = _build()
    return _COMPILED


def _make_vones_init():
    vo = np.zeros((128, NMC, 8, 128), np.float32)
    vo[:, :, 0::2, 64:128] = 1.0
    vo[:, :, 1::2, 0:64] = 1.0
    return vo


def kernel(x, cond, q_w, q_b, kv_w, kv_b, proj_w, proj_b):
    global LAST_RESULTS
    from concourse.bass_utils import run_bass_kernel_spmd

    x = np.asarray(x, np.float32)
    cond = np.asarray(cond, np.float32)
    q_w = np.asarray(q_w, np.float32)
    q_b = np.asarray(q_b, np.float32)
    kv_w = np.asarray(kv_w, np.float32)
    kv_b = np.asarray(kv_b, np.float32)
    proj_w = np.asarray(proj_w, np.float32)
    proj_b = np.asarray(proj_b, np.float32)

    nc = _get_compiled()
    vones = _make_vones_init()

    in_maps = []
    for c in range(N_CORES):
        b, hh = c // 2, c % 2
        cs = slice(hh * 512, (hh + 1) * 512)
        in_maps.append({
            "xT": np.ascontiguousarray(x[b].T),
            "condT": np.ascontiguousarray(cond[b].T),
            "qw": np.ascontiguousarray(q_w[:, cs]),
            "kwk": np.ascontiguousarray(kv_w[:, hh * 512:(hh + 1) * 512]),
            "kwv": np.ascontiguousarray(kv_w[:, C + hh * 512:C + (hh + 1) * 512]),
            "pw": np.ascontiguousarray(proj_w[cs, :]),
            "qb": np.ascontiguousarray(q_b[cs]),
            "kb": np.ascontiguousarray(kv_b[hh * 512:(hh + 1) * 512]),
            "vones_init": vones,
        })

    res = run_bass_kernel_spmd(nc, in_maps, core_ids=list(range(N_CORES)))
    LAST_RESULTS = res

    # host reduction: sum the two head-half partials per batch, add the
    # folded biases (proj_b and the v-bias contribution kv_b_v @ proj_w).
    bias = proj_b.astype(np.float64) + kv_b[C:].astype(np.float64) @ proj_w.astype(np.float64)
    out = np.empty((B, N, C), np.float32)
    for b in range(B):
        acc = res.results[2 * b]["out"].astype(np.float64)
        acc += res.results[2 * b + 1]["out"].astype(np.float64)
        acc += bias
        out[b] = acc.astype(np.float32)
    return out


# revision 15
# speedup vs baseline: 1.0963x; 1.0963x over previous
"""Cross-attention kernel for Trainium2, 8 NeuronCores.

Problem (hardcoded): x[4,2048,1024], cond[4,1024,768], dim=1024, cond_dim=768,
H=16 heads, hd=64.  out = proj(softmax(q k^T / sqrt(hd)) v) + proj_b.

Sharding: Megatron-style hybrid — batch (4) x head-half (2) = 8 shards.
Core c handles batch b=c//2 and heads [8*(c%2), 8*(c%2)+8).  Each core
computes its 8 heads' attention and a partial projection output [2048,1024];
the host sums the two partials per batch and adds the biases folded out of
the device program (proj_b and the v-bias term kv_b_v @ proj_w).

Device math per core (all matmuls in float32r, 1 PE cycle/row):
  qT  = (x W_q + q_b)^T          [512, 2048]   (pairs of heads = 128-row chunks)
  kT  = (cond W_k + k_b)^T       [512, 1024]
  v   =  cond W_v                [1024, 512]   (v-bias folded to host)
  per head pair p, query span s (512 tokens), cond chunk mc (128 tokens):
    scores^T = row-packed pair of K=64 matmuls -> psum [128 m, 2 heads, 512 nq]
    exp      = one ACT op: exp(0.125 * scores) -> f32r
    AV+sums  = K=128 matmul with lhsT = [v_h | ones] (even) / [ones | v_h] (odd)
               accumulated over mc -> [64 av rows, 64 sum rows]
    out_h    = av * reciprocal(sums)
  partial = outT^T @ W_p_slice   [2048, 1024]  (accumulate 4 pairs in psum)
"""

import sys

if '/opt/trn_rl_repo' not in sys.path:
    sys.path.insert(0, '/opt/trn_rl_repo')

import numpy as np

B, N, C = 4, 2048, 1024
CONDN, CONDC = 1024, 768
H, HD = 16, 64
N_CORES = 8
SCALE = HD ** -0.5

NPAIR = 4            # head pairs per core (8 heads)
NSPAN = 4            # query spans of 512
SPAN = 512
NMC = 8              # cond chunks of 128
KC_Q = 8             # contraction chunks for q proj (1024/128)
KC_KV = 6            # contraction chunks for kv proj (768/128)

_COMPILED = None
LAST_RESULTS = None


def _build():
    import concourse.bacc as bacc
    import concourse.mybir as mybir
    from concourse import tile

    F32R = mybir.dt.float32r
    F32 = mybir.dt.float32
    MULT = mybir.AluOpType.mult
    EXP = mybir.ActivationFunctionType.Exp

    nc = bacc.Bacc("TRN2", target_bir_lowering=False, num_devices=N_CORES)

    xT_d = nc.dram_tensor("xT", [C, N], F32R, kind="ExternalInput")
    condT_d = nc.dram_tensor("condT", [CONDC, CONDN], F32R, kind="ExternalInput")
    qw_d = nc.dram_tensor("qw", [C, 512], F32R, kind="ExternalInput")
    kwk_d = nc.dram_tensor("kwk", [CONDC, 512], F32R, kind="ExternalInput")
    kwv_d = nc.dram_tensor("kwv", [CONDC, 512], F32R, kind="ExternalInput")
    pw_d = nc.dram_tensor("pw", [512, C], F32R, kind="ExternalInput")
    qb_d = nc.dram_tensor("qb", [512], F32, kind="ExternalInput")
    kb_d = nc.dram_tensor("kb", [512], F32, kind="ExternalInput")
    vones_d = nc.dram_tensor("vones_init", [128, NMC, 8, 128], F32R, kind="ExternalInput")
    out_d = nc.dram_tensor("out", [N, C], F32, kind="ExternalOutput")

    with tile.TileContext(nc) as tc:
        with (
            tc.tile_pool(name="const", bufs=1) as const,
            tc.tile_pool(name="xt", bufs=16) as xt_pool,
            tc.tile_pool(name="ct", bufs=6) as ct_pool,
            tc.tile_pool(name="qt", bufs=2) as qt_pool,
            tc.tile_pool(name="ex", bufs=3) as ex_pool,
            tc.tile_pool(name="ot", bufs=2) as ot_pool,
            tc.tile_pool(name="rc", bufs=2) as rc_pool,
            tc.tile_pool(name="ob", bufs=3) as ob_pool,
            tc.tile_pool(name="psA", bufs=2, space="PSUM") as psA,
            tc.tile_pool(name="psB", bufs=2, space="PSUM") as psB,
            tc.tile_pool(name="psC", bufs=2, space="PSUM") as psC,
        ):
            # ---- persistent weights / constants -------------------------------
            # DMA emission order = consumption order, so nothing early waits
            # on a large transfer it does not need: kv weights first (KV phase
            # runs first), vones chunks (needed mid-KV-phase), q weights
            # (Q-proj after KV), proj weights last (first use is in span 1).
            kwk_sb = const.tile([128, KC_KV, 512], F32R)
            nc.sync.dma_start(kwk_sb[:], kwk_d.ap().rearrange("(kc p) m -> p kc m", p=128))
            kb_sb = const.tile([128, NPAIR], F32)
            nc.sync.dma_start(kb_sb[:], kb_d.ap().rearrange("(pp p) -> p pp", p=128))
            kwv_sb = const.tile([128, KC_KV, 512], F32R)
            nc.sync.dma_start(kwv_sb[:], kwv_d.ap().rearrange("(kc p) m -> p kc m", p=128))
            vones = const.tile([128, NMC, 8, 128], F32R)
            for mc in range(NMC):
                nc.sync.dma_start(vones[:, mc], vones_d.ap()[:, mc])
            qw_sb = const.tile([128, KC_Q, 512], F32R)
            qb_sb = const.tile([128, NPAIR], F32)
            pw_sb = const.tile([128, NPAIR, C], F32R)
            kT_sb = const.tile([128, NPAIR, NMC, 128], F32R)

            # ---- KV phase: kT = (cond Wk + kb)^T, v = cond Wv ----------------
            for ms in range(2):  # cond-token spans of 512
                cts = []
                for kc in range(KC_KV):
                    ct = ct_pool.tile([128, 512], F32R, name="ct")
                    nc.sync.dma_start(
                        ct[:],
                        condT_d.ap()[kc * 128:(kc + 1) * 128, ms * 512:(ms + 1) * 512],
                    )
                    cts.append(ct)
                for p in range(NPAIR):
                    ps = psC.tile([128, 512], F32, tag="C", name="kps")
                    for kc in range(KC_KV):
                        nc.tensor.matmul(
                            ps[:], kwk_sb[:, kc, p * 128:(p + 1) * 128], cts[kc][:],
                            start=(kc == 0), stop=(kc == KC_KV - 1),
                        )
                    nc.vector.tensor_scalar_add(
                        kT_sb[:, p, ms * 4:(ms + 1) * 4, :], ps[:], kb_sb[:, p:p + 1],
                    )
                for mj in range(4):  # token chunks of 128 within span
                    mc = ms * 4 + mj
                    ps = psC.tile([128, 512], F32, tag="C", name="vps")
                    for kc in range(KC_KV):
                        nc.tensor.matmul(
                            ps[:], cts[kc][:, mj * 128:(mj + 1) * 128], kwv_sb[:, kc, :],
                            start=(kc == 0), stop=(kc == KC_KV - 1),
                        )
                    # scatter v into [v|1]/[1|v] interleave: even heads cols 0:64,
                    # odd heads cols 64:128
                    ps_v = ps.rearrange("q (h d) -> q h d", d=64)
                    nc.vector.tensor_copy(vones[:, mc, 0::2, 0:64], ps_v[:, 0::2, :])
                    nc.vector.tensor_copy(vones[:, mc, 1::2, 64:128], ps_v[:, 1::2, :])

            # ---- main loop over query spans ----------------------------------
            # PE stream is kept dense three ways: AV matmuls trail their QK by
            # one cond-chunk (so the exp they wait on has a chunk of slack),
            # Q-proj groups for span s+1 and output-proj groups for span s-1
            # are spread into span s's attention loop as filler work.
            def dma_xts(s):
                xts = []
                for kc in range(KC_Q):
                    xt = xt_pool.tile([128, 512], F32R, name="xt")
                    nc.sync.dma_start(
                        xt[:],
                        xT_d.ap()[kc * 128:(kc + 1) * 128, s * SPAN:(s + 1) * SPAN],
                    )
                    xts.append(xt)
                return xts

            def qproj_pair(qt, xts, p):
                """Returns a list of single-matmul thunks (filler granularity)."""
                cell = {}

                def mm(kc):
                    if kc == 0:
                        cell["ps"] = psC.tile([128, 512], F32, tag="C", name="qps")
                    nc.tensor.matmul(
                        cell["ps"][:], qw_sb[:, kc, p * 128:(p + 1) * 128], xts[kc][:],
                        start=(kc == 0), stop=(kc == KC_Q - 1),
                    )
                    if kc == KC_Q - 1:
                        nc.vector.tensor_scalar_add(
                            qt[:, p, :], cell["ps"][:], qb_sb[:, p:p + 1],
                        )

                return [lambda kc=kc: mm(kc) for kc in range(KC_Q)]

            def proj_group(s, ot, t, o):
                cell = {}

                def mm(p):
                    if p == 0:
                        cell["pp"] = psC.tile([128, 512], F32, tag="C", name="pp")
                    nc.tensor.matmul(
                        cell["pp"][:], ot[:, p, t * 128:(t + 1) * 128],
                        pw_sb[:, p, o * 512:(o + 1) * 512],
                        start=(p == 0), stop=(p == NPAIR - 1),
                    )
                    if p == NPAIR - 1:
                        ob = ob_pool.tile([128, 512], F32, name="ob")
                        nc.vector.tensor_copy(ob[:], cell["pp"][:])
                        nc.sync.dma_start(
                            out_d.ap()[s * SPAN + t * 128:s * SPAN + (t + 1) * 128,
                                       o * 512:(o + 1) * 512],
                            ob[:],
                        )

                return [lambda p=p: mm(p) for p in range(NPAIR)]

            nc.sync.dma_start(qw_sb[:], qw_d.ap().rearrange("(kc p) m -> p kc m", p=128))
            nc.sync.dma_start(qb_sb[:], qb_d.ap().rearrange("(pp p) -> p pp", p=128))
            xts = dma_xts(0)
            qt = qt_pool.tile([128, NPAIR, SPAN], F32R, name="qt")
            for p in range(NPAIR):
                for th in qproj_pair(qt, xts, p):
                    th()
            nc.sync.dma_start(pw_sb[:], pw_d.ap().rearrange("(pp p) o -> p pp o", p=128))

            prev_ot = None
            for s in range(NSPAN):
                # filler matmuls to interleave into this span's attention loop
                # (one thunk per matmul): Q-proj for span s+1 and output-proj
                # for span s-1.
                filler = []
                if s + 1 < NSPAN:
                    next_xts = dma_xts(s + 1)
                    next_qt = qt_pool.tile([128, NPAIR, SPAN], F32R, name="qt")
                    for p in range(NPAIR):
                        filler.extend(qproj_pair(next_qt, next_xts, p))
                else:
                    next_xts = next_qt = None
                if prev_ot is not None:
                    for t in range(4):
                        for o in range(2):
                            filler.extend(proj_group(s - 1, prev_ot, t, o))
                filler.reverse()  # pop() from the front
                per_step = max(1, -(-len(filler) // 32))  # ceil over 32 mc-steps

                ot = ot_pool.tile([128, NPAIR, SPAN], F32R, name="ot")
                for p in range(NPAIR):
                    av = [psB.tile([128, 512], F32, tag="av", name=f"av{h}") for h in range(2)]
                    pend = []  # (mc, exp tile) entries awaiting their AV matmuls
                    for mc in range(NMC):
                        qk = psA.tile([128, 1024], F32, tag="A", name="qk")
                        qk2 = qk.rearrange("q (h n) -> q h n", h=2)
                        nc.tensor.matmul(
                            qk2[:, 0], kT_sb[0:64, p, mc, :], qt[0:64, p, :],
                            start=True, stop=True,
                        )
                        nc.tensor.matmul(
                            qk2[:, 1], kT_sb[64:128, p, mc, :], qt[64:128, p, :],
                            start=True, stop=True,
                        )
                        ex = ex_pool.tile([128, 2, 512], F32R, name="ex")
                        nc.scalar.activation(ex[:], qk2[:], EXP, scale=SCALE)
                        for _ in range(per_step):
                            if filler:
                                filler.pop()()
                        pend.append((mc, ex))
                        if len(pend) > 2:  # AV trails its QK by two cond-chunks
                            pmc, pex = pend.pop(0)
                            for h in range(2):
                                nc.tensor.matmul(
                                    av[h][:], vones[:, pmc, 2 * p + h, :], pex[:, h, :],
                                    start=(pmc == 0), stop=False,
                                )
                    for pmc, pex in pend:
                        for h in range(2):
                            nc.tensor.matmul(
                                av[h][:], vones[:, pmc, 2 * p + h, :], pex[:, h, :],
                                start=(pmc == 0), stop=(pmc == NMC - 1),
                            )
                    # normalize: even head -> av rows 0:64, sums 64:128;
                    #            odd head  -> sums 0:64, av 64:128
                    sums = rc_pool.tile([128, 512], F32, name="sums")
                    nc.vector.tensor_copy(sums[0:64, :], av[0][64:128, :])
                    nc.vector.tensor_copy(sums[64:128, :], av[1][0:64, :])
                    rcp = rc_pool.tile([128, 512], F32, name="rcp")
                    # approx_fast needs SBUF input (fp32 bit-trick seed), ~51 ULP
                    nc.vector.reciprocal_approx_fast(rcp[:], sums[:])
                    nc.vector.tensor_tensor(
                        ot[0:64, p, :], av[0][0:64, :], rcp[0:64, :], op=MULT,
                    )
                    nc.vector.tensor_tensor(
                        ot[64:128, p, :], av[1][64:128, :], rcp[64:128, :], op=MULT,
                    )
                while filler:
                    filler.pop()()
                prev_ot = ot
                qt = next_qt

            # trailing projection for the last span
            for t in range(4):
                for o in range(2):
                    for th in proj_group(NSPAN - 1, prev_ot, t, o):
                        th()

    nc.compile()
    return nc


def _get_compiled():
    global _COMPILED
    if _COMPILED is None:
        _COMPILED = _build()
    return _COMPILED


def _make_vones_init():
    vo = np.zeros((128, NMC, 8, 128), np.float32)
    vo[:, :, 0::2, 64:128] = 1.0
    vo[:, :, 1::2, 0:64] = 1.0
    return vo


def kernel(x, cond, q_w, q_b, kv_w, kv_b, proj_w, proj_b):
    global LAST_RESULTS
    from concourse.bass_utils import run_bass_kernel_spmd

    x = np.asarray(x, np.float32)
    cond = np.asarray(cond, np.float32)
    q_w = np.asarray(q_w, np.float32)
    q_b = np.asarray(q_b, np.float32)
    kv_w = np.asarray(kv_w, np.float32)
    kv_b = np.asarray(kv_b, np.float32)
    proj_w = np.asarray(proj_w, np.float32)
    proj_b = np.asarray(proj_b, np.float32)

    nc = _get_compiled()
    vones = _make_vones_init()

    in_maps = []
    for c in range(N_CORES):
        b, hh = c // 2, c % 2
        cs = slice(hh * 512, (hh + 1) * 512)
        in_maps.append({
            "xT": np.ascontiguousarray(x[b].T),
            "condT": np.ascontiguousarray(cond[b].T),
            "qw": np.ascontiguousarray(q_w[:, cs]),
            "kwk": np.ascontiguousarray(kv_w[:, hh * 512:(hh + 1) * 512]),
            "kwv": np.ascontiguousarray(kv_w[:, C + hh * 512:C + (hh + 1) * 512]),
            "pw": np.ascontiguousarray(proj_w[cs, :]),
            "qb": np.ascontiguousarray(q_b[cs]),
            "kb": np.ascontiguousarray(kv_b[hh * 512:(hh + 1) * 512]),
            "vones_init": vones,
        })

    res = run_bass_kernel_spmd(nc, in_maps, core_ids=list(range(N_CORES)))
    LAST_RESULTS = res

    # host reduction: sum the two head-half partials per batch, add the
    # folded biases (proj_b and the v-bias contribution kv_b_v @ proj_w).
    bias = proj_b.astype(np.float64) + kv_b[C:].astype(np.float64) @ proj_w.astype(np.float64)
    out = np.empty((B, N, C), np.float32)
    for b in range(B):
        acc = res.results[2 * b]["out"].astype(np.float64)
        acc += res.results[2 * b + 1]["out"].astype(np.float64)
        acc += bias
        out[b] = acc.astype(np.float32)
    return out


# revision 18
# speedup vs baseline: 1.1190x; 1.0207x over previous
"""Cross-attention kernel for Trainium2, 8 NeuronCores.

Problem (hardcoded): x[4,2048,1024], cond[4,1024,768], dim=1024, cond_dim=768,
H=16 heads, hd=64.  out = proj(softmax(q k^T / sqrt(hd)) v) + proj_b.

Sharding: Megatron-style hybrid — batch (4) x head-half (2) = 8 shards.
Core c handles batch b=c//2 and heads [8*(c%2), 8*(c%2)+8).  Each core
computes its 8 heads' attention and a partial projection output [2048,1024];
the host sums the two partials per batch and adds the biases folded out of
the device program (proj_b and the v-bias term kv_b_v @ proj_w).

Device math per core (all matmuls in float32r, 1 PE cycle/row):
  qT  = (x W_q + q_b)^T          [512, 2048]   (pairs of heads = 128-row chunks)
  kT  = (cond W_k + k_b)^T       [512, 1024]
  v   =  cond W_v                [1024, 512]   (v-bias folded to host)
  per head pair p, query span s (512 tokens), cond chunk mc (128 tokens):
    scores^T = row-packed pair of K=64 matmuls -> psum [128 m, 2 heads, 512 nq]
    exp      = one ACT op: exp(0.125 * scores) -> f32r
    AV+sums  = K=128 matmul with lhsT = [v_h | ones] (even) / [ones | v_h] (odd)
               accumulated over mc -> [64 av rows, 64 sum rows]
    out_h    = av * reciprocal(sums)
  partial = outT^T @ W_p_slice   [2048, 1024]  (accumulate 4 pairs in psum)
"""

import sys

if '/opt/trn_rl_repo' not in sys.path:
    sys.path.insert(0, '/opt/trn_rl_repo')

import numpy as np

B, N, C = 4, 2048, 1024
CONDN, CONDC = 1024, 768
H, HD = 16, 64
N_CORES = 8
SCALE = HD ** -0.5

NPAIR = 4            # head pairs per core (8 heads)
NSPAN = 4            # query spans of 512
SPAN = 512
NMC = 8              # cond chunks of 128
KC_Q = 8             # contraction chunks for q proj (1024/128)
KC_KV = 6            # contraction chunks for kv proj (768/128)

_COMPILED = None
LAST_RESULTS = None


def _build():
    import concourse.bacc as bacc
    import concourse.mybir as mybir
    from concourse import tile

    F32R = mybir.dt.float32r
    F32 = mybir.dt.float32
    MULT = mybir.AluOpType.mult
    EXP = mybir.ActivationFunctionType.Exp

    nc = bacc.Bacc("TRN2", target_bir_lowering=False, num_devices=N_CORES)

    xT_d = nc.dram_tensor("xT", [C, N], F32R, kind="ExternalInput")
    condT_d = nc.dram_tensor("condT", [CONDC, CONDN], F32R, kind="ExternalInput")
    qw_d = nc.dram_tensor("qw", [C, 512], F32R, kind="ExternalInput")
    kwk_d = nc.dram_tensor("kwk", [CONDC, 512], F32R, kind="ExternalInput")
    kwv_d = nc.dram_tensor("kwv", [CONDC, 512], F32R, kind="ExternalInput")
    pw_d = nc.dram_tensor("pw", [512, C], F32R, kind="ExternalInput")
    qb_d = nc.dram_tensor("qb", [512], F32, kind="ExternalInput")
    kb_d = nc.dram_tensor("kb", [512], F32, kind="ExternalInput")
    vones_d = nc.dram_tensor("vones_init", [128, NMC, 8, 128], F32R, kind="ExternalInput")
    out_d = nc.dram_tensor("out", [N, C], F32, kind="ExternalOutput")

    with tile.TileContext(nc) as tc:
        with (
            tc.tile_pool(name="const", bufs=1) as const,
            tc.tile_pool(name="xt", bufs=14) as xt_pool,
            tc.tile_pool(name="ct", bufs=6) as ct_pool,
            tc.tile_pool(name="qt", bufs=2) as qt_pool,
            tc.tile_pool(name="ex", bufs=4) as ex_pool,
            tc.tile_pool(name="ot", bufs=2) as ot_pool,
            tc.tile_pool(name="rc", bufs=2) as rc_pool,
            tc.tile_pool(name="ob", bufs=3) as ob_pool,
            tc.tile_pool(name="psA", bufs=2, space="PSUM") as psA,
            tc.tile_pool(name="psB", bufs=2, space="PSUM") as psB,
            tc.tile_pool(name="psC", bufs=2, space="PSUM") as psC,
        ):
            # ---- persistent weights / constants -------------------------------
            # DMA emission order = consumption order, so nothing early waits
            # on a large transfer it does not need: kv weights first (KV phase
            # runs first), vones chunks (needed mid-KV-phase), q weights
            # (Q-proj after KV), proj weights last (first use is in span 1).
            kwk_sb = const.tile([128, KC_KV, 512], F32R)
            nc.sync.dma_start(kwk_sb[:], kwk_d.ap().rearrange("(kc p) m -> p kc m", p=128))
            kb_sb = const.tile([128, NPAIR], F32)
            nc.sync.dma_start(kb_sb[:], kb_d.ap().rearrange("(pp p) -> p pp", p=128))
            vones = const.tile([128, NMC, 8, 128], F32R)
            qw_sb = const.tile([128, KC_Q, 512], F32R)
            qb_sb = const.tile([128, NPAIR], F32)
            pw_sb = const.tile([128, NPAIR, C], F32R)
            kT_sb = const.tile([128, NPAIR, NMC, 128], F32R)
            kwv_sb = const.tile([128, KC_KV, 512], F32R)

            def dma_cts(ms):
                cts = []
                for kc in range(KC_KV):
                    ct = ct_pool.tile([128, 512], F32R, name="ct")
                    nc.sync.dma_start(
                        ct[:],
                        condT_d.ap()[kc * 128:(kc + 1) * 128, ms * 512:(ms + 1) * 512],
                    )
                    cts.append(ct)
                return cts

            # first cond tiles ahead of the bulkier kwv/vones transfers, so the
            # first kT-proj matmul starts after ~3MB of DMA instead of ~10MB
            cts0 = dma_cts(0)
            nc.sync.dma_start(kwv_sb[:], kwv_d.ap().rearrange("(kc p) m -> p kc m", p=128))

            # ---- KV phase: kT = (cond Wk + kb)^T, v = cond Wv ----------------
            for ms in range(2):  # cond-token spans of 512
                cts = cts0 if ms == 0 else dma_cts(1)
                for p in range(NPAIR):
                    ps = psC.tile([128, 512], F32, tag="C", name="kps")
                    for kc in range(KC_KV):
                        nc.tensor.matmul(
                            ps[:], kwk_sb[:, kc, p * 128:(p + 1) * 128], cts[kc][:],
                            start=(kc == 0), stop=(kc == KC_KV - 1),
                        )
                    nc.vector.tensor_scalar_add(
                        kT_sb[:, p, ms * 4:(ms + 1) * 4, :], ps[:], kb_sb[:, p:p + 1],
                    )
                for mj in range(4):  # token chunks of 128 within span
                    mc = ms * 4 + mj
                    nc.sync.dma_start(vones[:, mc], vones_d.ap()[:, mc])
                    ps = psC.tile([128, 512], F32, tag="C", name="vps")
                    for kc in range(KC_KV):
                        nc.tensor.matmul(
                            ps[:], cts[kc][:, mj * 128:(mj + 1) * 128], kwv_sb[:, kc, :],
                            start=(kc == 0), stop=(kc == KC_KV - 1),
                        )
                    # scatter v into [v|1]/[1|v] interleave: even heads cols 0:64,
                    # odd heads cols 64:128
                    ps_v = ps.rearrange("q (h d) -> q h d", d=64)
                    nc.vector.tensor_copy(vones[:, mc, 0::2, 0:64], ps_v[:, 0::2, :])
                    nc.vector.tensor_copy(vones[:, mc, 1::2, 64:128], ps_v[:, 1::2, :])

            # ---- main loop over query spans ----------------------------------
            # PE stream is kept dense three ways: AV matmuls trail their QK by
            # one cond-chunk (so the exp they wait on has a chunk of slack),
            # Q-proj groups for span s+1 and output-proj groups for span s-1
            # are spread into span s's attention loop as filler work.
            def dma_xts(s):
                xts = []
                for kc in range(KC_Q):
                    xt = xt_pool.tile([128, 512], F32R, name="xt")
                    nc.sync.dma_start(
                        xt[:],
                        xT_d.ap()[kc * 128:(kc + 1) * 128, s * SPAN:(s + 1) * SPAN],
                    )
                    xts.append(xt)
                return xts

            def qproj_pair(qt, xts, p):
                """Returns a list of single-matmul thunks (filler granularity)."""
                cell = {}

                def mm(kc):
                    if kc == 0:
                        cell["ps"] = psC.tile([128, 512], F32, tag="C", name="qps")
                    nc.tensor.matmul(
                        cell["ps"][:], qw_sb[:, kc, p * 128:(p + 1) * 128], xts[kc][:],
                        start=(kc == 0), stop=(kc == KC_Q - 1),
                    )
                    if kc == KC_Q - 1:
                        nc.vector.tensor_scalar_add(
                            qt[:, p, :], cell["ps"][:], qb_sb[:, p:p + 1],
                        )

                return [lambda kc=kc: mm(kc) for kc in range(KC_Q)]

            def proj_group(s, ot, t, o):
                cell = {}

                def mm(p):
                    if p == 0:
                        cell["pp"] = psC.tile([128, 512], F32, tag="C", name="pp")
                    nc.tensor.matmul(
                        cell["pp"][:], ot[:, p, t * 128:(t + 1) * 128],
                        pw_sb[:, p, o * 512:(o + 1) * 512],
                        start=(p == 0), stop=(p == NPAIR - 1),
                    )
                    if p == NPAIR - 1:
                        ob = ob_pool.tile([128, 512], F32, name="ob")
                        nc.vector.tensor_copy(ob[:], cell["pp"][:])
                        nc.sync.dma_start(
                            out_d.ap()[s * SPAN + t * 128:s * SPAN + (t + 1) * 128,
                                       o * 512:(o + 1) * 512],
                            ob[:],
                        )

                return [lambda p=p: mm(p) for p in range(NPAIR)]

            nc.sync.dma_start(qw_sb[:], qw_d.ap().rearrange("(kc p) m -> p kc m", p=128))
            nc.sync.dma_start(qb_sb[:], qb_d.ap().rearrange("(pp p) -> p pp", p=128))
            xts = dma_xts(0)
            qt = qt_pool.tile([128, NPAIR, SPAN], F32R, name="qt")
            for p in range(NPAIR):
                for th in qproj_pair(qt, xts, p):
                    th()
            nc.sync.dma_start(pw_sb[:], pw_d.ap().rearrange("(pp p) o -> p pp o", p=128))

            prev_ot = None
            for s in range(NSPAN):
                # filler matmuls to interleave into this span's attention loop
                # (one thunk per matmul): Q-proj for span s+1 and output-proj
                # for span s-1.
                filler = []
                if s + 1 < NSPAN:
                    next_xts = dma_xts(s + 1)
                    next_qt = qt_pool.tile([128, NPAIR, SPAN], F32R, name="qt")
                    for p in range(NPAIR):
                        filler.extend(qproj_pair(next_qt, next_xts, p))
                else:
                    next_xts = next_qt = None
                if prev_ot is not None:
                    for t in range(4):
                        for o in range(2):
                            filler.extend(proj_group(s - 1, prev_ot, t, o))
                filler.reverse()  # pop() from the front
                per_step = max(1, -(-len(filler) // 32))  # ceil over 32 mc-steps

                ot = ot_pool.tile([128, NPAIR, SPAN], F32R, name="ot")
                for p in range(NPAIR):
                    av = [psB.tile([128, 512], F32, tag="av", name=f"av{h}") for h in range(2)]
                    pend = []  # (mc, exp tile) entries awaiting their AV matmuls
                    for mc in range(NMC):
                        qk = psA.tile([128, 1024], F32, tag="A", name="qk")
                        qk2 = qk.rearrange("q (h n) -> q h n", h=2)
                        nc.tensor.matmul(
                            qk2[:, 0], kT_sb[0:64, p, mc, :], qt[0:64, p, :],
                            start=True, stop=True,
                        )
                        nc.tensor.matmul(
                            qk2[:, 1], kT_sb[64:128, p, mc, :], qt[64:128, p, :],
                            start=True, stop=True,
                        )
                        ex = ex_pool.tile([128, 2, 512], F32R, name="ex")
                        nc.scalar.activation(ex[:], qk2[:], EXP, scale=SCALE)
                        for _ in range(per_step):
                            if filler:
                                filler.pop()()
                        pend.append((mc, ex))
                        if len(pend) > 2:  # AV trails its QK by two cond-chunks
                            pmc, pex = pend.pop(0)
                            for h in range(2):
                                nc.tensor.matmul(
                                    av[h][:], vones[:, pmc, 2 * p + h, :], pex[:, h, :],
                                    start=(pmc == 0), stop=False,
                                )
                    for pmc, pex in pend:
                        for h in range(2):
                            nc.tensor.matmul(
                                av[h][:], vones[:, pmc, 2 * p + h, :], pex[:, h, :],
                                start=(pmc == 0), stop=(pmc == NMC - 1),
                            )
                    # normalize: even head -> av rows 0:64, sums 64:128;
                    #            odd head  -> sums 0:64, av 64:128
                    sums = rc_pool.tile([128, 512], F32, name="sums")
                    nc.vector.tensor_copy(sums[0:64, :], av[0][64:128, :])
                    nc.vector.tensor_copy(sums[64:128, :], av[1][0:64, :])
                    rcp = rc_pool.tile([128, 512], F32, name="rcp")
                    # approx_fast needs SBUF input (fp32 bit-trick seed), ~51 ULP
                    nc.vector.reciprocal_approx_fast(rcp[:], sums[:])
                    nc.vector.tensor_tensor(
                        ot[0:64, p, :], av[0][0:64, :], rcp[0:64, :], op=MULT,
                    )
                    nc.vector.tensor_tensor(
                        ot[64:128, p, :], av[1][64:128, :], rcp[64:128, :], op=MULT,
                    )
                while filler:
                    filler.pop()()
                prev_ot = ot
                qt = next_qt

            # trailing projection for the last span
            for t in range(4):
                for o in range(2):
                    for th in proj_group(NSPAN - 1, prev_ot, t, o):
                        th()

    nc.compile()
    return nc


def _get_compiled():
    global _COMPILED
    if _COMPILED is None:
        _COMPILED = _build()
    return _COMPILED


def _make_vones_init():
    vo = np.zeros((128, NMC, 8, 128), np.float32)
    vo[:, :, 0::2, 64:128] = 1.0
    vo[:, :, 1::2, 0:64] = 1.0
    return vo


def kernel(x, cond, q_w, q_b, kv_w, kv_b, proj_w, proj_b):
    global LAST_RESULTS
    from concourse.bass_utils import run_bass_kernel_spmd

    x = np.asarray(x, np.float32)
    cond = np.asarray(cond, np.float32)
    q_w = np.asarray(q_w, np.float32)
    q_b = np.asarray(q_b, np.float32)
    kv_w = np.asarray(kv_w, np.float32)
    kv_b = np.asarray(kv_b, np.float32)
    proj_w = np.asarray(proj_w, np.float32)
    proj_b = np.asarray(proj_b, np.float32)

    nc = _get_compiled()
    vones = _make_vones_init()

    in_maps = []
    for c in range(N_CORES):
        b, hh = c // 2, c % 2
        cs = slice(hh * 512, (hh + 1) * 512)
        in_maps.append({
            "xT": np.ascontiguousarray(x[b].T),
            "condT": np.ascontiguousarray(cond[b].T),
            "qw": np.ascontiguousarray(q_w[:, cs]),
            "kwk": np.ascontiguousarray(kv_w[:, hh * 512:(hh + 1) * 512]),
            "kwv": np.ascontiguousarray(kv_w[:, C + hh * 512:C + (hh + 1) * 512]),
            "pw": np.ascontiguousarray(proj_w[cs, :]),
            "qb": np.ascontiguousarray(q_b[cs]),
            "kb": np.ascontiguousarray(kv_b[hh * 512:(hh + 1) * 512]),
            "vones_init": vones,
        })

    res = run_bass_kernel_spmd(nc, in_maps, core_ids=list(range(N_CORES)))
    LAST_RESULTS = res

    # host reduction: sum the two head-half partials per batch, add the
    # folded biases (proj_b and the v-bias contribution kv_b_v @ proj_w).
    bias = proj_b.astype(np.float64) + kv_b[C:].astype(np.float64) @ proj_w.astype(np.float64)
    out = np.empty((B, N, C), np.float32)
    for b in range(B):
        acc = res.results[2 * b]["out"].astype(np.float64)
        acc += res.results[2 * b + 1]["out"].astype(np.float64)
        acc += bias
        out[b] = acc.astype(np.float32)
    return out
